# revision 11
# baseline (speedup 1.0000x reference)
"""Grouped-query attention, tensor-parallel over heads across 8 TRN2 NeuronCores.

Problem (hardcoded): x[1,1024,4096] @ Wq/Wk/Wv -> RoPE -> causal GQA
(32 q heads, 8 kv groups, head_dim 128) -> out proj Wo -> [1,1024,4096].

Sharding: core r owns q heads 4r..4r+3 and kv group r (Wq/Wk/Wv column
shards, Wo row shard). Each core computes a full [1024,4096] partial of
the output projection; the host sums the 8 partials (the "all-reduce").

Device kernel (per core, all matmuls bf16 with fp32 PSUM accumulation):
  qT[hd,s] = sum_c Wq_c^T x_c      (transposed layouts throughout; the
  kT[hd,s], v[t,hd]                 host ships x pre-transposed so no
  RoPE via permutation-matmul + DVE  on-device transposes are needed)
  ST[t,s] = khat_tile^T qhat       (causal: skip fully-masked tiles)
  P = exp(ST) * mask01             (1/sqrt(128) folded into Wq on host)
  den[1,s] = ones^T P;  ctxT[hd,s] = sum_t v^T P
  ctxhat = ctxT * (1/den broadcast) (DRAM-bounce partition broadcast)
  out[s,:] += ctxhat_h^T Wo_h       (accumulate 4 heads in PSUM)
"""

import numpy as np
import ml_dtypes

import concourse.bass as bass
import concourse.bacc as bacc
import concourse.mybir as mybir
import concourse.tile as tile
from concourse.bass_utils import run_bass_kernel_spmd

S = 1024          # sequence length
D = 4096          # model dim
H = 32            # query heads (global)
G = 8             # kv groups (global)
HD = 128          # head dim
N_CORES = 8
HPC = H // N_CORES   # 4 query heads per core
QW = HPC * HD        # 512 q-proj cols per core
NDC = D // 128       # 32 contraction chunks
BF = mybir.dt.bfloat16
F32 = mybir.dt.float32

_CACHE = {}


def _t_tiles(j):
    """Causal t-tile list for the 512-wide s-tile j, with mask index or None."""
    out = []
    for i in range(4 * j + 4):
        lo = i - 4 * j          # 128*i <= 512*j + ls needs mask when i-4j in 0..3
        out.append((i, lo if 0 <= lo <= 3 else None))
    return out


def _build():
    nc = bacc.Bacc("TRN2", target_bir_lowering=False, debug=False,
                   num_devices=N_CORES)

    xT = nc.dram_tensor("xT", [128, NDC, S], BF, kind="ExternalInput")
    wq = nc.dram_tensor("wq", [128, NDC, QW], BF, kind="ExternalInput")
    wk = nc.dram_tensor("wk", [128, NDC, HD], BF, kind="ExternalInput")
    wv = nc.dram_tensor("wv", [128, NDC, HD], BF, kind="ExternalInput")
    wo = nc.dram_tensor("wo", [128, HPC, D], BF, kind="ExternalInput")
    cosT = nc.dram_tensor("cosT", [HD, S], BF, kind="ExternalInput")
    sinT = nc.dram_tensor("sinT", [HD, S], BF, kind="ExternalInput")
    rmat = nc.dram_tensor("rmat", [HD, HD], BF, kind="ExternalInput")
    masks = nc.dram_tensor("masks", [128, 4, 512], BF, kind="ExternalInput")
    out = nc.dram_tensor("out", [S, D], BF, kind="ExternalOutput")

    with tile.TileContext(nc) as tc:
        _emit(tc, nc, xT, wq, wk, wv, wo, cosT, sinT, rmat, masks, out)
    nc.compile()
    return nc


def _emit(tc, nc, xT, wq, wk, wv, wo, cosT, sinT, rmat, masks, out):
    import contextlib
    ctx = contextlib.ExitStack()
    with ctx:
        const = ctx.enter_context(tc.tile_pool(name="const", bufs=1))
        work = ctx.enter_context(tc.tile_pool(name="work", bufs=1))
        tmp = ctx.enter_context(tc.tile_pool(name="tmp", bufs=4))
        pt_pool = ctx.enter_context(tc.tile_pool(name="pt", bufs=9))
        outp = ctx.enter_context(tc.tile_pool(name="outp", bufs=4))
        ps = ctx.enter_context(tc.tile_pool(name="ps", bufs=8, space="PSUM"))
        dram = ctx.enter_context(tc.tile_pool(name="dramb", bufs=4, space="DRAM"))

        # ---- constants / weights into SBUF ----
        # Emission order == consumption order. wk is split across queues so
        # the very first k-matmul unblocks fast; cos/sin/wv/masks/wo queue
        # behind the x/wq stream (they are consumed later).
        rmat_sb = const.tile([HD, HD], BF, tag="rmat")
        nc.sync.dma_start(out=rmat_sb[:], in_=rmat.ap())
        ones_sb = const.tile([128, 1], BF, tag="ones")
        nc.vector.memset(ones_sb[:], 1.0)
        # Leading transfers kept small so the first k/q matmuls unblock
        # fast; later groups are bigger (HWDGE launch overhead is per-DMA).
        wk_sb = const.tile([128, NDC, HD], BF, tag="wk")
        nc.sync.dma_start(out=wk_sb[:, 0:4, :], in_=wk.ap()[:, 0:4, :])
        # chunk groups of [2,2,4,4,...]: first transfers small so the first
        # matmuls unblock early, later ones big to amortize launch overhead
        sizes = [2, 2] + [4] * 7
        gx = {}
        gw = {}
        off = 0
        for gi, sz in enumerate(sizes):
            g = const.tile([128, sz, S], BF, tag=f"xg{gi}", name=f"xg{gi}")
            nc.sync.dma_start(out=g[:], in_=xT.ap()[:, off:off + sz, :])
            for k in range(sz):
                gx[off + k] = g[:, k, :]
            g = const.tile([128, sz, QW], BF, tag=f"wqg{gi}", name=f"wqg{gi}")
            nc.sync.dma_start(out=g[:], in_=wq.ap()[:, off:off + sz, :])
            for k in range(sz):
                gw[off + k] = g[:, k, :]
            if gi == 0:  # rest of wk right after the first x/wq pair
                nc.sync.dma_start(out=wk_sb[:, 4:, :], in_=wk.ap()[:, 4:, :])
            off += sz
        x_sb = [gx[c] for c in range(NDC)]
        wq_sb = [gw[c] for c in range(NDC)]
        cos_sb = const.tile([HD, S], BF, tag="cos")
        nc.sync.dma_start(out=cos_sb[:], in_=cosT.ap())
        sin_sb = const.tile([HD, S], BF, tag="sin")
        nc.sync.dma_start(out=sin_sb[:], in_=sinT.ap())
        wv_sb = const.tile([128, NDC, HD], BF, tag="wv")
        nc.sync.dma_start(out=wv_sb[:], in_=wv.ap())
        mask_sb = const.tile([128, 4, 512], BF, tag="mask")
        nc.sync.dma_start(out=mask_sb[:], in_=masks.ap())
        wo_sb = const.tile([128, HPC, D], BF, tag="wo")
        for n in range(2):
            nc.sync.dma_start(out=wo_sb[:, :, n * 2048:(n + 1) * 2048],
                              in_=wo.ap()[:, :, n * 2048:(n + 1) * 2048])

        # persistent activations
        qhat = {}
        khat = {}
        for j in range(2):
            khat[j] = work.tile([HD, 512], BF, tag=f"khat{j}", name=f"khat{j}")
            for h in range(HPC):
                qhat[(h, j)] = work.tile([HD, 512], BF, tag=f"qhat{h}_{j}", name=f"qhat{h}_{j}")
        v_sb = [work.tile([128, HD], BF, tag=f"v{i}", name=f"v{i}") for i in range(8)]
        ctx_sb = {(h, j): work.tile([HD, 512], BF, tag=f"ctx{h}_{j}", name=f"ctx{h}_{j}")
                  for j in range(2) for h in range(HPC)}

        def rope_copy(src_psum):
            raw = tmp.tile([HD, 512], BF, tag="rope_raw", name="rope_raw", bufs=4)
            nc.scalar.activation(raw[:], src_psum[:],
                                 mybir.ActivationFunctionType.Copy)
            return raw

        def rope_rest(dst, raw, j):
            rq = ps.tile([HD, 512], F32, tag="ps", name="ps")
            nc.tensor.matmul(rq[:], rmat_sb[:], raw[:], start=True, stop=True)
            t1 = tmp.tile([HD, 512], BF, tag="rope_t1", name="rope_t1", bufs=2)
            nc.vector.tensor_mul(t1[:], raw[:], cos_sb[:, j * 512:(j + 1) * 512])
            t2 = tmp.tile([HD, 512], BF, tag="rope_t2", name="rope_t2", bufs=2)
            nc.vector.tensor_mul(t2[:], rq[:], sin_sb[:, j * 512:(j + 1) * 512])
            nc.vector.tensor_add(dst[:], t1[:], t2[:])

        # ---- QKV projections + RoPE (per s-half) ----
        # Chunk-major: the k-chain and all four q-chains advance together
        # per x-chunk, so PE starts as soon as chunk 0 lands and is paced
        # by compute, not by the x/wq DMA stream.
        for j in range(2):
            sl = slice(j * 512, (j + 1) * 512)
            kp = ps.tile([HD, 512], F32, tag="ps", name="ps")
            qps = [ps.tile([HD, 512], F32, tag="ps", name=f"qp{h}")
                   for h in range(HPC)]
            for c in range(NDC):
                nc.tensor.matmul(kp[:], wk_sb[:, c, :], x_sb[c][:, sl],
                                 start=(c == 0), stop=(c == NDC - 1))
                for h in range(HPC):
                    nc.tensor.matmul(qps[h][:], wq_sb[c][:, h * HD:(h + 1) * HD],
                                     x_sb[c][:, sl],
                                     start=(c == 0), stop=(c == NDC - 1))
            kraw = rope_copy(kp)
            qraws = [rope_copy(qps[h]) for h in range(HPC)]
            rope_rest(khat[j], kraw, j)
            for h in range(HPC):
                rope_rest(qhat[(h, j)], qraws[h], j)
        for i in range(8):
            vp = ps.tile([128, HD], F32, tag="ps", name="ps")
            for c in range(NDC):
                nc.tensor.matmul(vp[:], x_sb[c][:, i * 128:(i + 1) * 128],
                                 wv_sb[:, c, :],
                                 start=(c == 0), stop=(c == NDC - 1))
            nc.vector.tensor_copy(v_sb[i][:], vp[:])

        # ---- attention + out-proj, per s-half ----
        for j in range(2):
            for h in range(HPC):
                tt = _t_tiles(j)
                pts = []
                for n, (i, m) in enumerate(tt):
                    kj, ki = divmod(i, 4)
                    st = ps.tile([128, 512], F32, tag="ps", name="ps")
                    nc.tensor.matmul(st[:], khat[kj][:, ki * 128:(ki + 1) * 128],
                                     qhat[(h, j)][:], start=True, stop=True)
                    pt = pt_pool.tile([128, 512], BF, tag="pt", name="pt")
                    if m is None:
                        nc.scalar.activation(pt[:], st[:],
                                             mybir.ActivationFunctionType.Exp)
                    else:
                        et = tmp.tile([128, 512], BF, tag="exp_tmp", name="exp_tmp", bufs=2)
                        nc.scalar.activation(et[:], st[:],
                                             mybir.ActivationFunctionType.Exp)
                        nc.vector.tensor_mul(pt[:], et[:], mask_sb[:, m, :])
                    pts.append((i, pt))
                # denominator chain first so recip/broadcast overlaps ctx MMs
                den = ps.tile([1, 512], F32, tag="ps", name="ps")
                for n, (i, pt) in enumerate(pts):
                    nc.tensor.matmul(den[:], ones_sb[:], pt[:],
                                     start=(n == 0), stop=(n == len(pts) - 1))
                rec = tmp.tile([1, 512], F32, tag="rec", name="rec", bufs=2)
                nc.vector.reciprocal(rec[:], den[:])
                bc = tmp.tile([128, 512], F32, tag="bc", name="bc", bufs=2)
                nc.gpsimd.partition_broadcast(bc[:], rec[:])
                cx = ps.tile([HD, 512], F32, tag="ps", name="ps")
                for n, (i, pt) in enumerate(pts):
                    nc.tensor.matmul(cx[:], v_sb[i][:], pt[:],
                                     start=(n == 0), stop=(n == len(pts) - 1))
                nc.vector.tensor_mul(ctx_sb[(h, j)][:], cx[:], bc[:])

            # out-proj for the 4 token tiles of this half; two 512-wide psum
            # chains share one bf16 [128,1024] staging tile and one DMA
            for q in range(4):
                st_i = 4 * j + q
                for n2 in range(4):
                    ot = outp.tile([128, 1024], BF, tag="ot", name="ot")
                    for sub in range(2):
                        n = 2 * n2 + sub
                        op = ps.tile([128, 512], F32, tag="ps", name="ps")
                        for h in range(HPC):
                            nc.tensor.matmul(
                                op[:],
                                ctx_sb[(h, j)][:, q * 128:(q + 1) * 128],
                                wo_sb[:, h, n * 512:(n + 1) * 512],
                                start=(h == 0), stop=(h == HPC - 1))
                        nc.vector.tensor_copy(ot[:, sub * 512:(sub + 1) * 512],
                                              op[:])
                    nc.sync.dma_start(
                        out=out.ap()[st_i * 128:(st_i + 1) * 128,
                                     n2 * 1024:(n2 + 1) * 1024],
                        in_=ot[:])


def _prep_inputs(x, cos, sin, Wq, Wk, Wv, Wo):
    """Host-side shard + layout prep. Returns per-core input maps."""
    bf = ml_dtypes.bfloat16
    x2 = np.asarray(x, np.float32).reshape(S, D)
    xTh = np.ascontiguousarray(x2.T).reshape(NDC, 128, S).transpose(1, 0, 2)
    xTh = np.ascontiguousarray(xTh).astype(bf)

    cosT = np.ascontiguousarray(np.asarray(cos, np.float32).T).astype(bf)
    sinT = np.ascontiguousarray(np.asarray(sin, np.float32).T).astype(bf)

    rmat = np.zeros((HD, HD), np.float32)
    half = HD // 2
    rmat[np.arange(half), np.arange(half) + half] = 1.0
    rmat[np.arange(half) + half, np.arange(half)] = -1.0
    rmat = rmat.astype(bf)

    lt = np.arange(128)[:, None]
    ls = np.arange(512)[None, :]
    masks = np.stack([(lt + 128 * m <= ls) for m in range(4)], axis=0)
    masks = np.ascontiguousarray(masks.transpose(1, 0, 2)).astype(bf)  # [128,4,512]

    scale = 1.0 / np.sqrt(np.float32(HD))
    Wq = np.asarray(Wq, np.float32) * scale
    Wk = np.asarray(Wk, np.float32)
    Wv = np.asarray(Wv, np.float32)
    Wo = np.asarray(Wo, np.float32)

    def chunked(w):  # [D, m] -> [128, NDC, m]
        m = w.shape[1]
        return np.ascontiguousarray(
            w.reshape(NDC, 128, m).transpose(1, 0, 2)).astype(bf)

    in_maps = []
    for r in range(N_CORES):
        wq_r = chunked(Wq[:, r * QW:(r + 1) * QW])
        wk_r = chunked(Wk[:, r * HD:(r + 1) * HD])
        wv_r = chunked(Wv[:, r * HD:(r + 1) * HD])
        wo_r = np.ascontiguousarray(
            Wo[r * QW:(r + 1) * QW, :].reshape(HPC, 128, D)
            .transpose(1, 0, 2)).astype(bf)
        in_maps.append({
            "xT": xTh, "wq": wq_r, "wk": wk_r, "wv": wv_r, "wo": wo_r,
            "cosT": cosT, "sinT": sinT, "rmat": rmat, "masks": masks,
        })
    return in_maps


def get_nc():
    if "nc" not in _CACHE:
        _CACHE["nc"] = _build()
    return _CACHE["nc"]


def kernel(x, mask, cos, sin, Wq, Wk, Wv, Wo):
    nc = get_nc()
    in_maps = _prep_inputs(x, cos, sin, Wq, Wk, Wv, Wo)
    res = run_bass_kernel_spmd(nc, in_maps, core_ids=list(range(N_CORES)))
    acc = np.zeros((S, D), np.float32)
    for r in range(N_CORES):
        acc += res.results[r]["out"].astype(np.float32)
    return acc[None]


if __name__ == "__main__":
    rng = np.random.default_rng(0)
    xs = rng.standard_normal((1, S, D), dtype=np.float32)
    print("built:", get_nc() is not None)
